# revision 68
# baseline (speedup 1.0000x reference)
"""Multi-head causal attention with RoPE on 8 TRN2 NeuronCores.

Problem: B=2, T=2048, D=1024, H=16 heads (dh=64), fp32 I/O.
  q/k/v = x @ w{q,k,v}.T ; RoPE(q,k) ; causal softmax((q k^T)/sqrt(dh)) @ v ;
  out = concat_heads @ wo.T

Sharding (8 cores): head-parallel compute, token-striped output. Core c owns
heads {2c, 2c+1} for both batches: it projects q/k/v for its 128 output
features (weights column-sliced on the host) and runs causal attention for
its 4 (batch, head) units. Four small AllToAll collectives (one per quarter
of the attention work, issued as that quarter completes, overlapping comm
with compute) redistribute attention outputs so core c ends up with all 1024
features for its four 128-token chunks {c, c+8, c+16, c+24}; it then applies
the full output projection for those chunks. The host interleaves the chunks
back into the full output.

Layout notes:
 - Host pre-transposes x -> xT [D, B*T] so projections produce q^T/k^T
   directly (features on partitions), which is exactly the matmul operand
   layout attention needs (S^T = K Q^T; PV uses P^T as the moving operand).
 - RoPE pairs are de-interleaved on the host by permuting wq/wk rows within
   each head (16 re rows then 16 im rows per 32-row quadrant) so the rotation
   becomes whole-tile ops; the partner swap is a DVE stream_shuffle.
 - A ones column is appended to each V chunk so the softmax denominator
   falls out of the same PV matmul (row 64 of the accumulator). V-group slots
   are 128-aligned because DMA-transpose destinations need 64-col alignment.
 - The two heads' K=64 QK matmuls use PE row-groups 0-63 / 64-127 and run
   concurrently in the systolic array.
 - Softmax skips the running-max: |scores|/8 < ~6 for unit-variance inputs,
   exp is computed in fp32 and cannot overflow.
"""

import numpy as np
import ml_dtypes

import concourse.bacc as bacc
import concourse.tile as tile
import concourse.mybir as mybir
from concourse import bass_utils

BF16 = mybir.dt.bfloat16
F32 = mybir.dt.float32
AF = mybir.ActivationFunctionType

NCORES = 8
B, T, D, H = 2, 2048, 1024, 16
DH = D // H          # 64
HPC = H // NCORES    # 2 heads per core
FPC = DH * HPC       # 128 features per core
TOK = B * T          # 4096
TPC = TOK // NCORES  # 512 tokens per core (output shard)
KC = D // 128        # 8 contraction chunks
NT = T // 512        # 4 query tiles of 512 per batch
VG = 256             # cols per v-group: [v_h0(64) | 1 | pad | v_h1(64) | 1 | pad]

_COMPILED = None


def _build(debug_taps=False):
    nc = bacc.Bacc("TRN2", target_bir_lowering=False, debug=False, num_devices=NCORES)

    xT_d = nc.dram_tensor("xT", [D, TOK], BF16, kind="ExternalInput")
    wq_d = nc.dram_tensor("wqT", [D, FPC], BF16, kind="ExternalInput")
    wk_d = nc.dram_tensor("wkT", [D, FPC], BF16, kind="ExternalInput")
    wv_d = nc.dram_tensor("wvT", [D, FPC], BF16, kind="ExternalInput")
    wo_d = nc.dram_tensor("woT", [D, D], BF16, kind="ExternalInput")
    C_d = nc.dram_tensor("cosC", [128, T], BF16, kind="ExternalInput")
    S_d = nc.dram_tensor("sinS", [128, T], BF16, kind="ExternalInput")
    mask_d = nc.dram_tensor("mask", [128, 128], BF16, kind="ExternalInput")
    id_d = nc.dram_tensor("ident", [128, 128], BF16, kind="ExternalInput")
    sel_d = nc.dram_tensor("sel", [4, 4 * DH], BF16, kind="ExternalInput")
    out_d = nc.dram_tensor("out", [TPC, D], F32, kind="ExternalOutput")

    swap16 = list(range(16, 32)) + list(range(16))

    with tile.TileContext(nc) as tc:
        with (
            tc.tile_pool(name="sb", bufs=1) as sb,
            tc.tile_pool(name="ps", bufs=1, space="PSUM") as ps,
            tc.tile_pool(name="dram", bufs=1, space="DRAM") as dram,
        ):
            # ---- small resident inputs first (one batched DMA per tensor:
            # chunk kc of rows lands at column block kc) ----
            wq_sb = sb.tile([128, KC * FPC], BF16)
            wk_sb = sb.tile([128, KC * FPC], BF16)
            wv_sb = sb.tile([128, KC * FPC], BF16)
            # constants go on the scalar-engine DGE queue so xT owns the sync
            # queue from cycle 0 (shorter ramp); q-path constants first
            C_sb = sb.tile([128, T], BF16)
            S_sb = sb.tile([128, T], BF16)
            mask2_sb = sb.tile([128, 256], BF16)  # 0/1 causal mask, twice (one per head)
            id_sb = sb.tile([128, 128], BF16)
            for w_sb, w_d in ((wq_sb, wq_d), (wk_sb, wk_d), (wv_sb, wv_d)):
                nc.scalar.dma_start(
                    w_sb[:].rearrange("p (k c) -> p k c", k=KC),
                    w_d[:].rearrange("(k p) c -> p k c", p=128),
                )
                if w_sb is wq_sb:
                    nc.scalar.dma_start(C_sb[:], C_d[:])
                    nc.scalar.dma_start(S_sb[:], S_d[:])
            nc.scalar.dma_start(mask2_sb[:, 0:128], mask_d[:])
            nc.scalar.dma_start(mask2_sb[:, 128:256], mask_d[:])
            nc.scalar.dma_start(id_sb[:], id_d[:])
            sel_sb = sb.tile([4, 4 * DH], BF16)
            nc.scalar.dma_start(sel_sb[:], sel_d[:])

            # xT loaded in (batch, 512-token tile)-major order so the first
            # projection can start after one column block. Layout: token-block
            # (b,n) occupies a contiguous 4096-col span (k-chunk major inside),
            # so each load is one flat region and dependency tracking is exact.
            xT_sb = sb.tile([128, KC * TOK], BF16)
            for b in range(B):
                for n in range(NT):
                    col = b * T + 512 * n
                    blk = (NT * b + n) * (KC * 512)
                    nc.sync.dma_start(
                        xT_sb[:, blk : blk + KC * 512].rearrange("p (k t) -> p k t", k=KC),
                        xT_d[:, col : col + 512].rearrange("(k p) t -> p k t", p=128),
                    )
            wo_sb = sb.tile([128, KC * D], BF16)
            nc.sync.dma_start(
                wo_sb[:].rearrange("p (k c) -> p k c", k=KC),
                wo_d[:].rearrange("(k p) c -> p k c", p=128),
            )

            # ---- persistent intermediates ----
            qrot_sb = sb.tile([128, TOK], BF16)
            krot_sb = sb.tile([128, TOK], BF16)
            v1_sb = sb.tile([128, B * (T // 128) * VG], BF16)
            # only column 64 of each 128-col head slot must be 1.0 (the
            # softmax-denominator ones column); strided memset is ~64 elems/lane
            nc.gpsimd.memset(
                v1_sb[:].rearrange("p (g c) -> p g c", c=128)[:, :, 64:65], 1.0
            )

            # 4 AllToAll groups: group g carries global token chunks 8g+o to rank o
            a2a_in = [dram.tile([D, 128], BF16, name=f"a2ain{g}") for g in range(4)]
            a2a_out = [dram.tile([D, 128], BF16, name=f"a2aout{g}") for g in range(4)]

            def proj_tile(w_sb, b, n):
                pp = ps.tile([128, 512], F32, tag="proj", bufs=2, name=f"pp{b}{n}")
                blk = (NT * b + n) * (KC * 512)
                for kc in range(KC):
                    nc.tensor.matmul(
                        pp[:],
                        w_sb[:, kc * FPC : (kc + 1) * FPC],
                        xT_sb[:, blk + 512 * kc : blk + 512 * kc + 512],
                        start=(kc == 0),
                        stop=(kc == KC - 1),
                    )
                return pp

            def rope_tile(pp, dst_sb, b, n):
                # dst = pp*C + swap16(pp)*S, reading the projection psum directly
                swp = sb.tile([128, 512], F32, tag="swp", bufs=3, name=f"swp{b}{n}")
                nc.vector.stream_shuffle(swp[:], pp[:], swap16)
                t1 = sb.tile([128, 512], BF16, tag="t1", bufs=3, name=f"t1{b}{n}")
                nc.vector.tensor_mul(t1[:], pp[:], C_sb[:, 512 * n : 512 * n + 512])
                t2 = sb.tile([128, 512], BF16, tag="t2", bufs=3, name=f"t2{b}{n}")
                nc.vector.tensor_mul(t2[:], swp[:], S_sb[:, 512 * n : 512 * n + 512])
                nc.vector.tensor_add(
                    dst_sb[:, b * T + 512 * n : b * T + 512 * n + 512], t1[:], t2[:]
                )

            def v_tile(b, n):
                pp = proj_tile(wv_sb, b, n)
                vtt = sb.tile([128, 512], BF16, tag="vtt", bufs=2, name=f"vtt{b}{n}")
                nc.scalar.activation(vtt[:], pp[:], AF.Copy)
                use_pe_transpose = True
                for i in range(4):
                    g = VG * ((T // 128) * b + 4 * n + i)
                    if use_pe_transpose:
                        tp = ps.tile([128, 128], BF16, tag="proj", bufs=2, name=f"tp{b}{n}{i}")
                        nc.tensor.matmul(
                            tp[:],
                            vtt[:, 128 * i : 128 * i + 128],
                            id_sb[:],
                            is_transpose=True,
                            start=True,
                            stop=True,
                        )
                        for h in range(2):
                            nc.vector.tensor_copy(
                                v1_sb[:, g + 128 * h : g + 128 * h + 64],
                                tp[:, 64 * h : 64 * h + 64],
                            )
                    else:
                        for h in range(2):
                            nc.sync.dma_start(
                                v1_sb[:, g + 128 * h : g + 128 * h + 64],
                                vtt[64 * h : 64 * h + 64, 128 * i : 128 * i + 128],
                                transpose=True,
                            )

            def attn_core(b, j):
                """Both heads for (batch b, q-tile j). The two heads' S tiles
                live in one 2-bank psum pair (cols 0:512 / 512:1024) so mask
                and exp are single strided ops; QK pairs run in disjoint PE
                row groups; PV trails QK by one chunk so the in-order PE queue
                never waits on the current chunk's exp. Returns the two [65,512]
                f32 SBUF copies of the O accumulators (row 64 = softmax sums)."""
                ops = [
                    ps.tile([65, 512], F32, tag="opsum", bufs=2, name=f"op{b}{h}{j}")
                    for h in range(2)
                ]
                nch = 4 * j + 4

                def qk_exp(c):
                    diag = c - 4 * j
                    lo = 128 * diag if diag >= 0 else 0
                    sp = ps.tile(
                        [128, 1024], F32, tag="spsum", bufs=2, name=f"sp{b}{j}{c}"
                    )
                    spv = sp[:].rearrange("p (h t) -> p h t", h=2)
                    for h in range(2):
                        nc.tensor.matmul(
                            sp[:, 512 * h + lo : 512 * h + 512],
                            krot_sb[64 * h : 64 * h + 64, b * T + 128 * c : b * T + 128 * c + 128],
                            qrot_sb[
                                64 * h : 64 * h + 64,
                                b * T + 512 * j + lo : b * T + 512 * j + 512,
                            ],
                            start=True,
                            stop=True,
                        )
                    pt = sb.tile(
                        [128, 1024], BF16, tag="pt", bufs=4, name=f"pt{b}{j}{c}"
                    )
                    ptv = pt[:].rearrange("p (h t) -> p h t", h=2)
                    nc.scalar.activation(
                        ptv[:, :, lo:512], spv[:, :, lo:512], AF.Exp, scale=0.125
                    )
                    if diag >= 0:
                        # zero the upper triangle post-exp (keeps DVE off the
                        # psum S path; PV already trails by one chunk)
                        nc.vector.tensor_mul(
                            ptv[:, :, lo : lo + 128], ptv[:, :, lo : lo + 128],
                            mask2_sb[:].rearrange("p (h t) -> p h t", h=2),
                        )
                    return pt

                def pv(c, pt):
                    diag = c - 4 * j
                    lo = 128 * diag if diag >= 0 else 0
                    g = VG * ((T // 128) * b + c)
                    for h in range(2):
                        nc.tensor.matmul(
                            ops[h][:, lo:512],
                            v1_sb[:, g + 128 * h : g + 128 * h + 65],
                            pt[:, 512 * h + lo : 512 * h + 512],
                            start=(c == 0),
                            stop=(c == nch - 1),
                        )

                prev = None
                for c in range(nch):
                    cur = qk_exp(c)
                    if prev is not None:
                        pv(c - 1, prev)
                    prev = cur
                pv(nch - 1, prev)
                # single copy to SBUF releases each opsum bank right away
                o65s = []
                for h in range(2):
                    o65 = sb.tile([65, 512], F32, tag="o65", bufs=4, name=f"o65{b}{h}{j}")
                    nc.scalar.activation(o65[:], ops[h][:], AF.Copy)
                    o65s.append(o65)
                return o65s

            def attn_epilogue(b, jpair, o65_by_j):
                """Normalize two q-tiles' outputs (4 head-tiles) with one
                batched reciprocal, then stage into the a2a buffer."""
                sums = sb.tile([4, 512], F32, tag="sums", bufs=2, name=f"sums{b}{jpair[0]}")
                units = []
                for ji, j in enumerate(jpair):
                    for h in range(2):
                        units.append((j, h, o65_by_j[j][h]))
                for r, (j, h, o65) in enumerate(units):
                    nc.sync.dma_start(sums[r : r + 1, :], o65[64:65, :])
                rec4 = sb.tile([4, 512], F32, tag="rec4", bufs=2, name=f"rec4{b}{jpair[0]}")
                nc.vector.reciprocal(rec4[:], sums[:])
                recb4 = sb.tile([4, 512], BF16, tag="recb4", bufs=2, name=f"recb4{b}{jpair[0]}")
                nc.vector.tensor_copy(recb4[:], rec4[:])
                for r, (j, h, o65) in enumerate(units):
                    bps = ps.tile([64, 512], F32, tag="proj", bufs=2, name=f"bps{b}{j}{h}")
                    nc.tensor.matmul(
                        bps[:], sel_sb[:, DH * r : DH * r + DH], recb4[:], start=True, stop=True
                    )
                    onr = sb.tile([64, 512], BF16, tag="onr", bufs=4, name=f"onr{b}{j}{h}")
                    nc.vector.tensor_mul(onr[:], o65[0:64, :], bps[:])
                    for i in range(4):
                        m = 16 * b + 4 * j + i
                        o, g = m % 8, m // 8
                        nc.sync.dma_start(
                            a2a_in[g][128 * o + 64 * h : 128 * o + 64 * h + 64, :],
                            onr[:, 128 * i : 128 * i + 128],
                        )

            def a2a_call(g):
                nc.gpsimd.collective_compute(
                    "AllToAll",
                    mybir.AluOpType.bypass,
                    replica_groups=[list(range(NCORES))],
                    ins=[a2a_in[g].opt()],
                    outs=[a2a_out[g].opt()],
                )

            at_tiles = {}

            def load_at(g):
                at = sb.tile([128, KC * 128], BF16, tag="at", bufs=4, name=f"at{g}")
                nc.sync.dma_start(
                    at[:].rearrange("p (k c) -> p k c", k=KC),
                    a2a_out[g][:].rearrange("(k p) c -> p k c", p=128),
                )
                at_tiles[g] = at

            def final_group(g):
                """Output projection for my token chunk in a2a group g."""
                if g not in at_tiles:
                    load_at(g)
                at = at_tiles[g]
                for nh in range(2):
                    fp = ps.tile([128, 512], F32, tag="proj", bufs=2, name=f"fp{g}{nh}")
                    for kc in range(KC):
                        nc.tensor.matmul(
                            fp[:],
                            at[:, 128 * kc : 128 * kc + 128],
                            wo_sb[:, kc * D + 512 * nh : kc * D + 512 * nh + 512],
                            start=(kc == 0),
                            stop=(kc == KC - 1),
                        )
                    fo = sb.tile([128, 512], F32, tag="fo", bufs=2, name=f"fo{g}{nh}")
                    nc.scalar.activation(fo[:], fp[:], AF.Copy)
                    nc.sync.dma_start(
                        out_d[128 * g : 128 * g + 128, 512 * nh : 512 * nh + 512], fo[:]
                    )

            # Software pipeline: proj(n+1) is emitted before attn(n) so the
            # rope DVE chain hides behind projection matmuls and PE never
            # stalls at attention-tile boundaries. Finals are emitted one
            # quarter after their collective so the in-order PE queue never
            # waits on a collective.
            def proj_all(b, n):
                qp = proj_tile(wq_sb, b, n)
                rope_tile(qp, qrot_sb, b, n)
                kp = proj_tile(wk_sb, b, n)
                rope_tile(kp, krot_sb, b, n)
                v_tile(b, n)

            # Every epilogue (reciprocal chain) is sandwiched behind dense PE
            # work (a projection or a final) so the in-order PE queue never
            # drains while DVE normalizes; finals 0-2 run during later
            # quarters, only final_3 truly trails the last collective.
            # epilogues (reciprocal chains) are sandwiched behind projection
            # matmuls so the in-order PE queue always has independent work
            o0, o1 = {}, {}
            proj_all(0, 0)
            proj_all(0, 1)
            o0[0] = attn_core(0, 0)
            proj_all(0, 2)
            o0[1] = attn_core(0, 1)
            proj_all(0, 3)
            attn_epilogue(0, (0, 1), o0)
            a2a_call(0)
            o0[2] = attn_core(0, 2)
            o0[3] = attn_core(0, 3)
            proj_all(1, 0)
            attn_epilogue(0, (2, 3), o0)
            a2a_call(1)
            proj_all(1, 1)
            o1[0] = attn_core(1, 0)
            proj_all(1, 2)
            o1[1] = attn_core(1, 1)
            proj_all(1, 3)
            attn_epilogue(1, (0, 1), o1)
            a2a_call(2)
            load_at(0)
            load_at(1)
            o1[2] = attn_core(1, 2)
            o1[3] = attn_core(1, 3)
            attn_epilogue(1, (2, 3), o1)
            a2a_call(3)
            load_at(2)
            for g in range(4):
                final_group(g)

            if debug_taps:
                dbg_q = nc.dram_tensor("dbg_q", [128, TOK], BF16, kind="ExternalOutput")
                dbg_k = nc.dram_tensor("dbg_k", [128, TOK], BF16, kind="ExternalOutput")
                dbg_v1 = nc.dram_tensor(
                    "dbg_v1", [128, B * (T // 128) * VG], BF16, kind="ExternalOutput"
                )
                dbg_a2a = nc.dram_tensor("dbg_a2a", [D, TPC], BF16, kind="ExternalOutput")
                nc.sync.dma_start(dbg_q[:], qrot_sb[:])
                nc.sync.dma_start(dbg_k[:], krot_sb[:])
                nc.sync.dma_start(dbg_v1[:], v1_sb[:])
                for g in range(4):
                    nc.sync.dma_start(dbg_a2a[:, 128 * g : 128 * g + 128], a2a_in[g][:])

    nc.compile()
    return nc


def _get_compiled():
    global _COMPILED
    if _COMPILED is None:
        _COMPILED = _build()
    return _COMPILED


def _prep_in_maps(embedding_word, wq, wk, wv, wo):
    bf = ml_dtypes.bfloat16
    x = np.asarray(embedding_word, np.float32).reshape(TOK, D)
    xT = np.ascontiguousarray(x.T).astype(bf)
    woT = np.ascontiguousarray(np.asarray(wo, np.float32).T).astype(bf)

    # within-head row permutation: 16 re rows then 16 im rows per 32-row quadrant
    perm64 = [
        (2 * (16 * q + r) if r < 16 else 2 * (16 * q + (r - 16)) + 1)
        for q in range(2)
        for r in range(32)
    ]
    perm64 = np.asarray(perm64)

    freqs = 1.0 / (10000.0 ** (np.arange(0, DH, 2, dtype=np.float64) / DH))  # [32]
    ang = np.arange(T, dtype=np.float64)[:, None] * freqs[None, :]  # [T, 32]
    cos_t, sin_t = np.cos(ang), np.sin(ang)
    rows = np.arange(128)
    wh = rows % 64
    qd = wh // 32
    r32 = wh % 32
    dmap = 16 * qd + (r32 % 16)
    sign = np.where(r32 < 16, -1.0, 1.0)
    C = np.ascontiguousarray(cos_t[:, dmap].T).astype(bf)  # [128, T]
    S = np.ascontiguousarray((sin_t[:, dmap] * sign[None, :]).T).astype(bf)

    rr = np.arange(128)[:, None]
    cc = np.arange(128)[None, :]
    mask = np.where(cc >= rr, 1.0, 0.0).astype(ml_dtypes.bfloat16)
    ident = np.eye(128, dtype=np.float32).astype(bf)
    sel = np.zeros((4, 4 * DH), np.float32)
    for r in range(4):
        sel[r, DH * r : DH * r + DH] = 1.0
    sel = sel.astype(bf)

    wqf = np.asarray(wq, np.float32)
    wkf = np.asarray(wk, np.float32)
    wvf = np.asarray(wv, np.float32)

    in_maps = []
    for c in range(NCORES):
        rows_c = slice(FPC * c, FPC * c + FPC)
        wq_c = wqf[rows_c].reshape(HPC, DH, D)[:, perm64, :].reshape(FPC, D)
        wk_c = wkf[rows_c].reshape(HPC, DH, D)[:, perm64, :].reshape(FPC, D)
        wv_c = wvf[rows_c]
        in_maps.append(
            {
                "xT": xT,
                "wqT": np.ascontiguousarray(wq_c.T).astype(bf),
                "wkT": np.ascontiguousarray(wk_c.T).astype(bf),
                "wvT": np.ascontiguousarray(wv_c.T).astype(bf),
                "woT": woT,
                "cosC": C,
                "sinS": S,
                "mask": mask,
                "ident": ident,
                "sel": sel,
            }
        )
    return in_maps


def _unshard(core_outs):
    """core_outs[c] is [TPC, D] covering token chunks {c, 8+c, 16+c, 24+c}
    (row-blocks g=0..3). Interleave back to [B, T, D]."""
    a = np.stack(core_outs, axis=0)  # [8, TPC, D]
    a = a.reshape(NCORES, 4, 128, D).transpose(1, 0, 2, 3).reshape(TOK, D)
    return np.ascontiguousarray(a.reshape(B, T, D).astype(np.float32))


def kernel(embedding_word, wq, wk, wv, wo):
    nc = _get_compiled()
    in_maps = _prep_in_maps(embedding_word, wq, wk, wv, wo)
    res = bass_utils.run_bass_kernel_spmd(nc, in_maps, core_ids=list(range(NCORES)))
    return _unshard([res.results[c]["out"] for c in range(NCORES)])


# revision 69
# speedup vs baseline: 1.0501x; 1.0501x over previous
"""Multi-head causal attention with RoPE on 8 TRN2 NeuronCores.

Problem: B=2, T=2048, D=1024, H=16 heads (dh=64), fp32 I/O.
  q/k/v = x @ w{q,k,v}.T ; RoPE(q,k) ; causal softmax((q k^T)/sqrt(dh)) @ v ;
  out = concat_heads @ wo.T

Sharding (8 cores): head-parallel compute, token-striped output. Core c owns
heads {2c, 2c+1} for both batches: it projects q/k/v for its 128 output
features (weights column-sliced on the host) and runs causal attention for
its 4 (batch, head) units. Four small AllToAll collectives (one per quarter
of the attention work, issued as that quarter completes, overlapping comm
with compute) redistribute attention outputs so core c ends up with all 1024
features for its four 128-token chunks {c, c+8, c+16, c+24}; it then applies
the full output projection for those chunks. The host interleaves the chunks
back into the full output.

Layout notes:
 - Host pre-transposes x -> xT [D, B*T] so projections produce q^T/k^T
   directly (features on partitions), which is exactly the matmul operand
   layout attention needs (S^T = K Q^T; PV uses P^T as the moving operand).
 - RoPE pairs are de-interleaved on the host by permuting wq/wk rows within
   each head (16 re rows then 16 im rows per 32-row quadrant) so the rotation
   becomes whole-tile ops; the partner swap is a DVE stream_shuffle.
 - A ones column is appended to each V chunk so the softmax denominator
   falls out of the same PV matmul (row 64 of the accumulator). V-group slots
   are 128-aligned because DMA-transpose destinations need 64-col alignment.
 - The two heads' K=64 QK matmuls use PE row-groups 0-63 / 64-127 and run
   concurrently in the systolic array.
 - Softmax skips the running-max: |scores|/8 < ~6 for unit-variance inputs,
   exp is computed in fp32 and cannot overflow.
"""

import numpy as np
import ml_dtypes

import concourse.bacc as bacc
import concourse.tile as tile
import concourse.mybir as mybir
from concourse import bass_utils

BF16 = mybir.dt.bfloat16
F32 = mybir.dt.float32
AF = mybir.ActivationFunctionType

NCORES = 8
B, T, D, H = 2, 2048, 1024, 16
DH = D // H          # 64
HPC = H // NCORES    # 2 heads per core
FPC = DH * HPC       # 128 features per core
TOK = B * T          # 4096
TPC = TOK // NCORES  # 512 tokens per core (output shard)
KC = D // 128        # 8 contraction chunks
NT = T // 512        # 4 query tiles of 512 per batch
VG = 256             # cols per v-group: [v_h0(64) | 1 | pad | v_h1(64) | 1 | pad]

_COMPILED = None


def _build(debug_taps=False):
    nc = bacc.Bacc("TRN2", target_bir_lowering=False, debug=False, num_devices=NCORES)

    xT_d = nc.dram_tensor("xT", [D, TOK], BF16, kind="ExternalInput")
    wq_d = nc.dram_tensor("wqT", [D, FPC], BF16, kind="ExternalInput")
    wk_d = nc.dram_tensor("wkT", [D, FPC], BF16, kind="ExternalInput")
    wv_d = nc.dram_tensor("wvT", [D, FPC], BF16, kind="ExternalInput")
    wo_d = nc.dram_tensor("woT", [D, D], BF16, kind="ExternalInput")
    C_d = nc.dram_tensor("cosC", [128, T], BF16, kind="ExternalInput")
    S_d = nc.dram_tensor("sinS", [128, T], BF16, kind="ExternalInput")
    mask_d = nc.dram_tensor("mask", [128, 128], BF16, kind="ExternalInput")
    id_d = nc.dram_tensor("ident", [128, 128], BF16, kind="ExternalInput")
    sel_d = nc.dram_tensor("sel", [4, 4 * DH], BF16, kind="ExternalInput")
    out_d = nc.dram_tensor("out", [TPC, D], F32, kind="ExternalOutput")

    swap16 = list(range(16, 32)) + list(range(16))

    with tile.TileContext(nc) as tc:
        with (
            tc.tile_pool(name="sb", bufs=1) as sb,
            tc.tile_pool(name="ps", bufs=1, space="PSUM") as ps,
            tc.tile_pool(name="dram", bufs=1, space="DRAM") as dram,
        ):
            # ---- small resident inputs first (one batched DMA per tensor:
            # chunk kc of rows lands at column block kc) ----
            wq_sb = sb.tile([128, KC * FPC], BF16)
            wk_sb = sb.tile([128, KC * FPC], BF16)
            wv_sb = sb.tile([128, KC * FPC], BF16)
            # constants go on the scalar-engine DGE queue so xT owns the sync
            # queue from cycle 0 (shorter ramp); q-path constants first
            C_sb = sb.tile([128, T], BF16)
            S_sb = sb.tile([128, T], BF16)
            mask2_sb = sb.tile([128, 256], BF16)  # 0/1 causal mask, twice (one per head)
            id_sb = sb.tile([128, 128], BF16)
            for w_sb, w_d in ((wq_sb, wq_d), (wk_sb, wk_d), (wv_sb, wv_d)):
                nc.scalar.dma_start(
                    w_sb[:].rearrange("p (k c) -> p k c", k=KC),
                    w_d[:].rearrange("(k p) c -> p k c", p=128),
                )
                if w_sb is wq_sb:
                    nc.scalar.dma_start(C_sb[:], C_d[:])
                    nc.scalar.dma_start(S_sb[:], S_d[:])
            nc.scalar.dma_start(mask2_sb[:, 0:128], mask_d[:])
            nc.scalar.dma_start(mask2_sb[:, 128:256], mask_d[:])
            nc.scalar.dma_start(id_sb[:], id_d[:])
            sel_sb = sb.tile([4, 4 * DH], BF16)
            nc.scalar.dma_start(sel_sb[:], sel_d[:])

            # xT loaded in (batch, 512-token tile)-major order so the first
            # projection can start after one column block. Layout: token-block
            # (b,n) occupies a contiguous 4096-col span (k-chunk major inside),
            # so each load is one flat region and dependency tracking is exact.
            xT_sb = sb.tile([128, KC * TOK], BF16)
            for b in range(B):
                for n in range(NT):
                    col = b * T + 512 * n
                    blk = (NT * b + n) * (KC * 512)
                    nc.sync.dma_start(
                        xT_sb[:, blk : blk + KC * 512].rearrange("p (k t) -> p k t", k=KC),
                        xT_d[:, col : col + 512].rearrange("(k p) t -> p k t", p=128),
                    )
            wo_sb = sb.tile([128, KC * D], BF16)
            nc.sync.dma_start(
                wo_sb[:].rearrange("p (k c) -> p k c", k=KC),
                wo_d[:].rearrange("(k p) c -> p k c", p=128),
            )

            # ---- persistent intermediates ----
            qrot_sb = sb.tile([128, TOK], BF16)
            krot_sb = sb.tile([128, TOK], BF16)
            v1_sb = sb.tile([128, B * (T // 128) * VG], BF16)
            # only column 64 of each 128-col head slot must be 1.0 (the
            # softmax-denominator ones column); strided memset is ~64 elems/lane
            nc.gpsimd.memset(
                v1_sb[:].rearrange("p (g c) -> p g c", c=128)[:, :, 64:65], 1.0
            )

            # 4 AllToAll groups: group g carries global token chunks 8g+o to rank o
            a2a_in = [dram.tile([D, 128], BF16, name=f"a2ain{g}") for g in range(4)]
            a2a_out = [dram.tile([D, 128], BF16, name=f"a2aout{g}") for g in range(4)]

            def proj_tile(w_sb, b, n):
                pp = ps.tile([128, 512], F32, tag="proj", bufs=2, name=f"pp{b}{n}")
                blk = (NT * b + n) * (KC * 512)
                for kc in range(KC):
                    nc.tensor.matmul(
                        pp[:],
                        w_sb[:, kc * FPC : (kc + 1) * FPC],
                        xT_sb[:, blk + 512 * kc : blk + 512 * kc + 512],
                        start=(kc == 0),
                        stop=(kc == KC - 1),
                    )
                return pp

            def rope_tile(pp, dst_sb, b, n):
                # dst = pp*C + swap16(pp)*S, reading the projection psum directly
                swp = sb.tile([128, 512], F32, tag="swp", bufs=3, name=f"swp{b}{n}")
                nc.vector.stream_shuffle(swp[:], pp[:], swap16)
                t1 = sb.tile([128, 512], BF16, tag="t1", bufs=3, name=f"t1{b}{n}")
                nc.vector.tensor_mul(t1[:], pp[:], C_sb[:, 512 * n : 512 * n + 512])
                t2 = sb.tile([128, 512], BF16, tag="t2", bufs=3, name=f"t2{b}{n}")
                nc.vector.tensor_mul(t2[:], swp[:], S_sb[:, 512 * n : 512 * n + 512])
                nc.vector.tensor_add(
                    dst_sb[:, b * T + 512 * n : b * T + 512 * n + 512], t1[:], t2[:]
                )

            def v_tile(b, n):
                pp = proj_tile(wv_sb, b, n)
                vtt = sb.tile([128, 512], BF16, tag="vtt", bufs=2, name=f"vtt{b}{n}")
                nc.scalar.activation(vtt[:], pp[:], AF.Copy)
                use_pe_transpose = True
                for i in range(4):
                    g = VG * ((T // 128) * b + 4 * n + i)
                    if use_pe_transpose:
                        tp = ps.tile([128, 128], BF16, tag="proj", bufs=2, name=f"tp{b}{n}{i}")
                        nc.tensor.matmul(
                            tp[:],
                            vtt[:, 128 * i : 128 * i + 128],
                            id_sb[:],
                            is_transpose=True,
                            start=True,
                            stop=True,
                        )
                        for h in range(2):
                            nc.vector.tensor_copy(
                                v1_sb[:, g + 128 * h : g + 128 * h + 64],
                                tp[:, 64 * h : 64 * h + 64],
                            )
                    else:
                        for h in range(2):
                            nc.sync.dma_start(
                                v1_sb[:, g + 128 * h : g + 128 * h + 64],
                                vtt[64 * h : 64 * h + 64, 128 * i : 128 * i + 128],
                                transpose=True,
                            )

            def attn_core(b, j):
                """Both heads for (batch b, q-tile j). The two heads' S tiles
                live in one 2-bank psum pair (cols 0:512 / 512:1024) so mask
                and exp are single strided ops; QK pairs run in disjoint PE
                row groups; PV trails QK by one chunk so the in-order PE queue
                never waits on the current chunk's exp. Returns the two [65,512]
                f32 SBUF copies of the O accumulators (row 64 = softmax sums)."""
                ops = [
                    ps.tile([65, 512], F32, tag="opsum", bufs=2, name=f"op{b}{h}{j}")
                    for h in range(2)
                ]
                nch = 4 * j + 4

                def qk_exp(c):
                    diag = c - 4 * j
                    lo = 128 * diag if diag >= 0 else 0
                    sp = ps.tile(
                        [128, 1024], F32, tag="spsum", bufs=2, name=f"sp{b}{j}{c}"
                    )
                    spv = sp[:].rearrange("p (h t) -> p h t", h=2)
                    for h in range(2):
                        nc.tensor.matmul(
                            sp[:, 512 * h + lo : 512 * h + 512],
                            krot_sb[64 * h : 64 * h + 64, b * T + 128 * c : b * T + 128 * c + 128],
                            qrot_sb[
                                64 * h : 64 * h + 64,
                                b * T + 512 * j + lo : b * T + 512 * j + 512,
                            ],
                            start=True,
                            stop=True,
                        )
                    pt = sb.tile(
                        [128, 1024], BF16, tag="pt", bufs=4, name=f"pt{b}{j}{c}"
                    )
                    ptv = pt[:].rearrange("p (h t) -> p h t", h=2)
                    nc.scalar.activation(
                        ptv[:, :, lo:512], spv[:, :, lo:512], AF.Exp, scale=0.125
                    )
                    if diag >= 0:
                        # zero the upper triangle post-exp (keeps DVE off the
                        # psum S path; PV already trails by one chunk)
                        nc.vector.tensor_mul(
                            ptv[:, :, lo : lo + 128], ptv[:, :, lo : lo + 128],
                            mask2_sb[:].rearrange("p (h t) -> p h t", h=2),
                        )
                    return pt

                def pv(c, pt):
                    diag = c - 4 * j
                    lo = 128 * diag if diag >= 0 else 0
                    g = VG * ((T // 128) * b + c)
                    for h in range(2):
                        nc.tensor.matmul(
                            ops[h][:, lo:512],
                            v1_sb[:, g + 128 * h : g + 128 * h + 65],
                            pt[:, 512 * h + lo : 512 * h + 512],
                            start=(c == 0),
                            stop=(c == nch - 1),
                        )

                prev = None
                for c in range(nch):
                    cur = qk_exp(c)
                    if prev is not None:
                        pv(c - 1, prev)
                    prev = cur
                pv(nch - 1, prev)
                # single copy to SBUF releases each opsum bank right away
                o65s = []
                for h in range(2):
                    o65 = sb.tile([65, 512], F32, tag="o65", bufs=4, name=f"o65{b}{h}{j}")
                    nc.scalar.activation(o65[:], ops[h][:], AF.Copy)
                    o65s.append(o65)
                return o65s

            def attn_epilogue(b, jpair, o65_by_j):
                """Normalize two q-tiles' outputs (4 head-tiles) with one
                batched reciprocal, then stage into the a2a buffer."""
                sums = sb.tile([4, 512], F32, tag="sums", bufs=2, name=f"sums{b}{jpair[0]}")
                units = []
                for ji, j in enumerate(jpair):
                    for h in range(2):
                        units.append((j, h, o65_by_j[j][h]))
                for r, (j, h, o65) in enumerate(units):
                    nc.sync.dma_start(sums[r : r + 1, :], o65[64:65, :])
                rec4 = sb.tile([4, 512], F32, tag="rec4", bufs=2, name=f"rec4{b}{jpair[0]}")
                nc.vector.reciprocal(rec4[:], sums[:])
                recb4 = sb.tile([4, 512], BF16, tag="recb4", bufs=2, name=f"recb4{b}{jpair[0]}")
                nc.vector.tensor_copy(recb4[:], rec4[:])
                for r, (j, h, o65) in enumerate(units):
                    bps = ps.tile([64, 512], F32, tag="proj", bufs=2, name=f"bps{b}{j}{h}")
                    nc.tensor.matmul(
                        bps[:], sel_sb[:, DH * r : DH * r + DH], recb4[:], start=True, stop=True
                    )
                    onr = sb.tile([64, 512], BF16, tag="onr", bufs=4, name=f"onr{b}{j}{h}")
                    nc.vector.tensor_mul(onr[:], o65[0:64, :], bps[:])
                    for i in range(4):
                        m = 16 * b + 4 * j + i
                        o, g = m % 8, m // 8
                        nc.sync.dma_start(
                            a2a_in[g][128 * o + 64 * h : 128 * o + 64 * h + 64, :],
                            onr[:, 128 * i : 128 * i + 128],
                        )

            def a2a_call(g):
                nc.gpsimd.collective_compute(
                    "AllToAll",
                    mybir.AluOpType.bypass,
                    replica_groups=[list(range(NCORES))],
                    ins=[a2a_in[g].opt()],
                    outs=[a2a_out[g].opt()],
                )

            at_tiles = {}

            def load_at(g):
                # scalar DGE queue: idle at the tail, so this starts the
                # moment collective g completes instead of draining sync first
                at = sb.tile([128, KC * 128], BF16, tag="at", bufs=4, name=f"at{g}")
                nc.scalar.dma_start(
                    at[:].rearrange("p (k c) -> p k c", k=KC),
                    a2a_out[g][:].rearrange("(k p) c -> p k c", p=128),
                )
                at_tiles[g] = at

            def final_group(g):
                """Output projection for my token chunk in a2a group g."""
                if g not in at_tiles:
                    load_at(g)
                at = at_tiles[g]
                for nh in range(2):
                    fp = ps.tile([128, 512], F32, tag="proj", bufs=2, name=f"fp{g}{nh}")
                    for kc in range(KC):
                        nc.tensor.matmul(
                            fp[:],
                            at[:, 128 * kc : 128 * kc + 128],
                            wo_sb[:, kc * D + 512 * nh : kc * D + 512 * nh + 512],
                            start=(kc == 0),
                            stop=(kc == KC - 1),
                        )
                    fo = sb.tile([128, 512], F32, tag="fo", bufs=2, name=f"fo{g}{nh}")
                    nc.scalar.activation(fo[:], fp[:], AF.Copy)
                    nc.sync.dma_start(
                        out_d[128 * g : 128 * g + 128, 512 * nh : 512 * nh + 512], fo[:]
                    )

            # Software pipeline: proj(n+1) is emitted before attn(n) so the
            # rope DVE chain hides behind projection matmuls and PE never
            # stalls at attention-tile boundaries. Finals are emitted one
            # quarter after their collective so the in-order PE queue never
            # waits on a collective.
            def proj_all(b, n):
                qp = proj_tile(wq_sb, b, n)
                rope_tile(qp, qrot_sb, b, n)
                kp = proj_tile(wk_sb, b, n)
                rope_tile(kp, krot_sb, b, n)
                v_tile(b, n)

            # Every epilogue (reciprocal chain) is sandwiched behind dense PE
            # work (a projection or a final) so the in-order PE queue never
            # drains while DVE normalizes; finals 0-2 run during later
            # quarters, only final_3 truly trails the last collective.
            # epilogues (reciprocal chains) are sandwiched behind projection
            # matmuls so the in-order PE queue always has independent work
            o0, o1 = {}, {}
            proj_all(0, 0)
            proj_all(0, 1)
            o0[0] = attn_core(0, 0)
            proj_all(0, 2)
            o0[1] = attn_core(0, 1)
            proj_all(0, 3)
            attn_epilogue(0, (0, 1), o0)
            a2a_call(0)
            o0[2] = attn_core(0, 2)
            o0[3] = attn_core(0, 3)
            proj_all(1, 0)
            attn_epilogue(0, (2, 3), o0)
            a2a_call(1)
            proj_all(1, 1)
            o1[0] = attn_core(1, 0)
            proj_all(1, 2)
            o1[1] = attn_core(1, 1)
            proj_all(1, 3)
            attn_epilogue(1, (0, 1), o1)
            a2a_call(2)
            load_at(0)
            load_at(1)
            o1[2] = attn_core(1, 2)
            o1[3] = attn_core(1, 3)
            attn_epilogue(1, (2, 3), o1)
            a2a_call(3)
            load_at(2)
            for g in range(4):
                final_group(g)

            if debug_taps:
                dbg_q = nc.dram_tensor("dbg_q", [128, TOK], BF16, kind="ExternalOutput")
                dbg_k = nc.dram_tensor("dbg_k", [128, TOK], BF16, kind="ExternalOutput")
                dbg_v1 = nc.dram_tensor(
                    "dbg_v1", [128, B * (T // 128) * VG], BF16, kind="ExternalOutput"
                )
                dbg_a2a = nc.dram_tensor("dbg_a2a", [D, TPC], BF16, kind="ExternalOutput")
                nc.sync.dma_start(dbg_q[:], qrot_sb[:])
                nc.sync.dma_start(dbg_k[:], krot_sb[:])
                nc.sync.dma_start(dbg_v1[:], v1_sb[:])
                for g in range(4):
                    nc.sync.dma_start(dbg_a2a[:, 128 * g : 128 * g + 128], a2a_in[g][:])

    nc.compile()
    return nc


def _get_compiled():
    global _COMPILED
    if _COMPILED is None:
        _COMPILED = _build()
    return _COMPILED


def _prep_in_maps(embedding_word, wq, wk, wv, wo):
    bf = ml_dtypes.bfloat16
    x = np.asarray(embedding_word, np.float32).reshape(TOK, D)
    xT = np.ascontiguousarray(x.T).astype(bf)
    woT = np.ascontiguousarray(np.asarray(wo, np.float32).T).astype(bf)

    # within-head row permutation: 16 re rows then 16 im rows per 32-row quadrant
    perm64 = [
        (2 * (16 * q + r) if r < 16 else 2 * (16 * q + (r - 16)) + 1)
        for q in range(2)
        for r in range(32)
    ]
    perm64 = np.asarray(perm64)

    freqs = 1.0 / (10000.0 ** (np.arange(0, DH, 2, dtype=np.float64) / DH))  # [32]
    ang = np.arange(T, dtype=np.float64)[:, None] * freqs[None, :]  # [T, 32]
    cos_t, sin_t = np.cos(ang), np.sin(ang)
    rows = np.arange(128)
    wh = rows % 64
    qd = wh // 32
    r32 = wh % 32
    dmap = 16 * qd + (r32 % 16)
    sign = np.where(r32 < 16, -1.0, 1.0)
    C = np.ascontiguousarray(cos_t[:, dmap].T).astype(bf)  # [128, T]
    S = np.ascontiguousarray((sin_t[:, dmap] * sign[None, :]).T).astype(bf)

    rr = np.arange(128)[:, None]
    cc = np.arange(128)[None, :]
    mask = np.where(cc >= rr, 1.0, 0.0).astype(ml_dtypes.bfloat16)
    ident = np.eye(128, dtype=np.float32).astype(bf)
    sel = np.zeros((4, 4 * DH), np.float32)
    for r in range(4):
        sel[r, DH * r : DH * r + DH] = 1.0
    sel = sel.astype(bf)

    wqf = np.asarray(wq, np.float32)
    wkf = np.asarray(wk, np.float32)
    wvf = np.asarray(wv, np.float32)

    in_maps = []
    for c in range(NCORES):
        rows_c = slice(FPC * c, FPC * c + FPC)
        wq_c = wqf[rows_c].reshape(HPC, DH, D)[:, perm64, :].reshape(FPC, D)
        wk_c = wkf[rows_c].reshape(HPC, DH, D)[:, perm64, :].reshape(FPC, D)
        wv_c = wvf[rows_c]
        in_maps.append(
            {
                "xT": xT,
                "wqT": np.ascontiguousarray(wq_c.T).astype(bf),
                "wkT": np.ascontiguousarray(wk_c.T).astype(bf),
                "wvT": np.ascontiguousarray(wv_c.T).astype(bf),
                "woT": woT,
                "cosC": C,
                "sinS": S,
                "mask": mask,
                "ident": ident,
                "sel": sel,
            }
        )
    return in_maps


def _unshard(core_outs):
    """core_outs[c] is [TPC, D] covering token chunks {c, 8+c, 16+c, 24+c}
    (row-blocks g=0..3). Interleave back to [B, T, D]."""
    a = np.stack(core_outs, axis=0)  # [8, TPC, D]
    a = a.reshape(NCORES, 4, 128, D).transpose(1, 0, 2, 3).reshape(TOK, D)
    return np.ascontiguousarray(a.reshape(B, T, D).astype(np.float32))


def kernel(embedding_word, wq, wk, wv, wo):
    nc = _get_compiled()
    in_maps = _prep_in_maps(embedding_word, wq, wk, wv, wo)
    res = bass_utils.run_bass_kernel_spmd(nc, in_maps, core_ids=list(range(NCORES)))
    return _unshard([res.results[c]["out"] for c in range(NCORES)])
